# revision 14
# baseline (speedup 1.0000x reference)
"""Trainium2 Bass kernel for nn_AttentionBlock (S=4096, H=1024, NH=2, DS=64).

Strategy: sequence parallelism over queries (512 per core on 8 cores).
K/V projections are replicated on every core (cheaper than collectives here).
All matmuls run in float32r (full PE rate, ~1e-4 relative rounding).
"""

import math
import sys

sys.path.insert(0, "/opt/trn_rl_repo")

import numpy as np

import concourse.bass as bass
import concourse.mybir as mybir
import concourse.tile as tile
from concourse import bacc
from concourse.bass_utils import run_bass_kernel_spmd

S, H, NH, DS = 4096, 1024, 2, 64
HD = H // NH            # 512
NC = 8                  # cores
SQ = S // NC            # 512 queries per core
EPS = 1e-5
F32 = mybir.dt.float32
F32R = mybir.dt.float32r
AF = mybir.ActivationFunctionType
ALU = mybir.AluOpType

KC = S // 128           # 32 key chunks of 128
HC = H // 128           # 8 hidden chunks of 128
QB = SQ // 128          # 4 query chunks of 128


def build_program(debug=False):
    nc = bacc.Bacc("TRN2", target_bir_lowering=False, debug=False, num_devices=NC)

    # ---- DRAM I/O ----
    x = nc.dram_tensor("x", [S, H], F32, kind="ExternalInput")
    xq = nc.dram_tensor("xq", [SQ, H], F32, kind="ExternalInput")
    wqT = nc.dram_tensor("wqT", [H, H], F32R, kind="ExternalInput")
    wkT = nc.dram_tensor("wkT", [H, H], F32R, kind="ExternalInput")
    wvT = nc.dram_tensor("wvT", [H, H], F32R, kind="ExternalInput")
    woT = nc.dram_tensor("woT", [H, H], F32R, kind="ExternalInput")
    wsT = nc.dram_tensor("wsT", [DS, H], F32R, kind="ExternalInput")
    sdat = nc.dram_tensor("sdat", [DS, 1], F32R, kind="ExternalInput")
    bsv = nc.dram_tensor("bsv", [H], F32, kind="ExternalInput")
    mbias = nc.dram_tensor("mbias", [S], F32, kind="ExternalInput")
    onescol = nc.dram_tensor("onescol", [128, 1], F32R, kind="ExternalInput")
    identd = nc.dram_tensor("identd", [128, 128], F32R, kind="ExternalInput")
    lnw = nc.dram_tensor("lnw", [H], F32, kind="ExternalInput")
    lnb = nc.dram_tensor("lnb", [H], F32, kind="ExternalInput")
    out = nc.dram_tensor("out", [SQ, H], F32, kind="ExternalOutput")
    if debug:
        dsemb = nc.dram_tensor("dsemb", [128, HC], F32, kind="ExternalOutput")
        dkbias = nc.dram_tensor("dkbias", [128, HC], F32, kind="ExternalOutput")
        dvb = nc.dram_tensor("dvb", [1, H], F32, kind="ExternalOutput")
        dxT = nc.dram_tensor("dxT", [128, 512], F32, kind="ExternalOutput")
        dqT = nc.dram_tensor("dqT", [128, 512], F32, kind="ExternalOutput")
        dPT = nc.dram_tensor("dPT", [128, 512], F32, kind="ExternalOutput")
        dl = nc.dram_tensor("dl", [NH, SQ], F32, kind="ExternalOutput")
        dctx = nc.dram_tensor("dctx", [128, 512], F32, kind="ExternalOutput")
        doutT = nc.dram_tensor("doutT", [128, 512], F32, kind="ExternalOutput")
        dKT = nc.dram_tensor("dKT", [128, 4, 512], F32, kind="ExternalOutput")
        dST = nc.dram_tensor("dST", [128, 512], F32, kind="ExternalOutput")

    inv_sqrt_hd = 1.0 / math.sqrt(HD)

    with tile.TileContext(nc) as tc:
        with (
            tc.tile_pool(name="consts", bufs=1) as consts,
            tc.tile_pool(name="persist", bufs=1) as persist,
            tc.tile_pool(name="dram", bufs=1, space="DRAM") as dram,
        ):
            # ---- constants ----
            ident = consts.tile([128, 128], F32R)
            nc.sync.dma_start(ident, identd[:, :])
            ones_sb = consts.tile([128, 1], F32R)
            nc.sync.dma_start(ones_sb, onescol[:, :])
            mb_sb = consts.tile([128, KC], F32)
            nc.sync.dma_start(mb_sb, mbias.rearrange("(c p) -> p c", p=128))
            zb_sb = consts.tile([128, 1], F32)
            nc.vector.memset(zb_sb, 0.0)
            eps_sb = consts.tile([128, 1], F32)
            nc.vector.memset(eps_sb, EPS)
            bs_row = consts.tile([1, H], F32)
            nc.sync.dma_start(bs_row, bsv.rearrange("d -> () d"))
            wsT_sb = consts.tile([DS, H], F32R)
            nc.sync.dma_start(wsT_sb, wsT[:, :])
            sd_sb = consts.tile([DS, 1], F32R)
            nc.sync.dma_start(sd_sb, sdat[:, :])

            # persistent across stages
            qT_sb = persist.tile([128, HC, SQ], F32R)      # Q^T/sqrt(hd): [d, q]
            ctx_sb = persist.tile([128, HC, SQ], F32R)     # ctx^T/l: [d, q]
            semb_pc = persist.tile([128, HC], F32R)        # static embedding [hin_p, hc]
            kbias_sb = persist.tile([128, HC], F32)

            # DRAM scratch
            kT_d = dram.tile([HC, 128, S], F32R)           # K^T as [dc, d_in_chunk, k]
            v_d = dram.tile([S, H], F32R)                  # V natural [k, d]
            vb_scr = dram.tile([H], F32)
            semb_scr = dram.tile([H], F32R)
            kb_scr = dram.tile([H], F32)
            l_scr = dram.tile([NH, SQ], F32)

            # ================= Stage 1: projections (eight key-blocks) =================
            SH = S // 8      # 512 keys per block
            KH = SH // 128   # 4 key chunks per block
            with (
                tc.tile_pool(name="xtp", bufs=2) as xtp,
                tc.tile_pool(name="w1", bufs=2) as w1,
                tc.tile_pool(name="ps1", bufs=3, space="PSUM") as ps1,
                tc.tile_pool(name="pst", bufs=3, space="PSUM") as pst,
                tc.tile_pool(name="psb", bufs=2, space="PSUM") as psb,
            ):
                vb_bcast = xtp.tile([128, H], F32, tag="vbb", bufs=1)
                # --- semb = Ws @ static + bs (row layout, then roundtrip) ---
                semb_row = xtp.tile([1, H], F32R, tag="srow", bufs=1)
                for d2 in range(H // 512):
                    p = psb.tile([1, 512], F32, tag="pbias", name=f"sembp{d2}")
                    nc.tensor.matmul(p[:], sd_sb[:], wsT_sb[:, d2 * 512:(d2 + 1) * 512],
                                     start=True, stop=True)
                    nc.vector.tensor_add(semb_row[:, d2 * 512:(d2 + 1) * 512], p[:],
                                         bs_row[:, d2 * 512:(d2 + 1) * 512])
                nc.sync.dma_start(semb_scr.rearrange("d -> () d"), semb_row[:])
                nc.sync.dma_start(semb_pc, semb_scr.rearrange("(c p) -> p c", p=128))

                # --- xq transpose + Q^T (scaled) first: shortens path to attention ---
                wq_sb = w1.tile([128, HC, H], F32R, tag="w", name="wq")
                nc.sync.dma_start(wq_sb, wqT.rearrange("(c p) d -> p c d", p=128))
                xqT_sb = xtp.tile([128, HC, SQ], F32R, tag="xT", name="xqT")
                for qb in range(QB):
                    xin = xtp.tile([128, H], F32R, tag="xin", name=f"xqin{qb}")
                    nc.sync.dma_start(xin, xq[qb * 128:(qb + 1) * 128, :].bitcast(F32R))
                    for hc in range(HC):
                        pt = pst.tile([128, 128], F32R, tag="ptr", name=f"qtr{qb}_{hc}")
                        nc.tensor.transpose(pt[:], xin[:, hc * 128:(hc + 1) * 128], ident[:])
                        nc.any.tensor_copy(xqT_sb[:, hc, qb * 128:(qb + 1) * 128], pt[:])
                for dc in range(HC):
                    p = ps1.tile([128, SQ], F32, tag="pproj", name=f"qp{dc}")
                    for hc in range(HC):
                        nc.tensor.matmul(p[:], wq_sb[:, hc, dc * 128:(dc + 1) * 128],
                                         xqT_sb[:, hc, :],
                                         start=(hc == 0), stop=(hc == HC - 1))
                    nc.scalar.mul(qT_sb[:, dc, :], p[:], inv_sqrt_hd)
                if debug:
                    nc.sync.dma_start(dqT[:, :], qT_sb[:, 0, :].bitcast(F32))

                wk_sb = w1.tile([128, HC, H], F32R, tag="w", name="wk")
                nc.sync.dma_start(wk_sb, wkT.rearrange("(c p) d -> p c d", p=128))
                wv_sb = w1.tile([128, HC, H], F32R, tag="w2", name="wv", bufs=1)
                nc.sync.dma_start(wv_sb, wvT.rearrange("(c p) d -> p c d", p=128))

                for quarter in range(8):
                    k0 = quarter * KH       # first 128-chunk of this quarter
                    # --- transpose x rows of this quarter -> xT_sb [128, HC, SH] ---
                    xT_sb = xtp.tile([128, HC, SH], F32R, tag="xT", name=f"xT{quarter}")
                    for kb in range(KH):
                        xin = xtp.tile([128, H], F32R, tag="xin",
                                         name=f"xin{quarter}_{kb}")
                        nc.sync.dma_start(xin, x[(k0 + kb) * 128:(k0 + kb + 1) * 128, :]
                                          .bitcast(F32R))
                        for hc in range(HC):
                            pt = pst.tile([128, 128], F32R, tag="ptr",
                                          name=f"ptr{quarter}_{kb}_{hc}")
                            nc.tensor.transpose(pt[:], xin[:, hc * 128:(hc + 1) * 128],
                                                ident[:])
                            nc.any.tensor_copy(xT_sb[:, hc, kb * 128:(kb + 1) * 128], pt[:])

                    if debug and quarter == 0:
                        nc.sync.dma_start(dxT[:, :], xT_sb[:, 0, 0:512].bitcast(F32))
                    # --- K^T for this quarter's key columns ---
                    if quarter == 0:
                        kb_row = xtp.tile([1, H], F32, tag="krow", bufs=1)
                        for d2 in range(H // 512):
                            p = psb.tile([1, 512], F32, tag="pbias", name=f"kbp{d2}")
                            for hc in range(HC):
                                nc.tensor.matmul(p[:], semb_pc[:, hc:hc + 1],
                                                 wk_sb[:, hc, d2 * 512:(d2 + 1) * 512],
                                                 start=(hc == 0), stop=(hc == HC - 1))
                            nc.vector.tensor_copy(kb_row[:, d2 * 512:(d2 + 1) * 512], p[:])
                        nc.sync.dma_start(kb_scr.rearrange("d -> () d"), kb_row[:])
                        nc.sync.dma_start(kbias_sb, kb_scr.rearrange("(c p) -> p c", p=128))
                        vb_row = xtp.tile([1, H], F32, tag="vrow", bufs=1)
                        for d2 in range(H // 512):
                            p = psb.tile([1, 512], F32, tag="pbias", name=f"vbp{d2}")
                            for hc in range(HC):
                                nc.tensor.matmul(p[:], semb_pc[:, hc:hc + 1],
                                                 wv_sb[:, hc, d2 * 512:(d2 + 1) * 512],
                                                 start=(hc == 0), stop=(hc == HC - 1))
                            nc.vector.tensor_copy(vb_row[:, d2 * 512:(d2 + 1) * 512], p[:])
                        nc.sync.dma_start(vb_scr.rearrange("d -> () d"), vb_row[:])
                        nc.sync.dma_start(vb_bcast,
                                          bass.AP(tensor=vb_scr.tensor, offset=vb_scr.offset,
                                                  ap=[[0, 128], [1, H]]))
                    for dc in range(HC):
                        for k2 in range(SH // 512):
                            p = ps1.tile([128, 512], F32, tag="pproj",
                                         name=f"kp{quarter}_{dc}_{k2}")
                            for hc in range(HC):
                                nc.tensor.matmul(p[:], wk_sb[:, hc, dc * 128:(dc + 1) * 128],
                                                 xT_sb[:, hc, k2 * 512:(k2 + 1) * 512],
                                                 start=(hc == 0), stop=(hc == HC - 1))
                            st = xtp.tile([128, 512], F32R, tag="kst",
                                            name=f"kst{quarter}_{dc}_{k2}")
                            nc.scalar.activation(st[:], p[:], AF.Identity,
                                                 bias=kbias_sb[:, dc:dc + 1])
                            nc.sync.dma_start(
                                kT_d[dc, :,
                                     quarter * SH + k2 * 512:quarter * SH + (k2 + 1) * 512],
                                st[:])

                    # --- V for this quarter's key rows ---
                    for kb in range(KH):
                        for d2 in range(H // 512):
                            p = ps1.tile([128, 512], F32, tag="pproj",
                                         name=f"vp{quarter}_{kb}_{d2}")
                            for hc in range(HC):
                                nc.tensor.matmul(p[:], xT_sb[:, hc, kb * 128:(kb + 1) * 128],
                                                 wv_sb[:, hc, d2 * 512:(d2 + 1) * 512],
                                                 start=(hc == 0), stop=(hc == HC - 1))
                            st = xtp.tile([128, 512], F32R, tag="vst",
                                            name=f"vst{quarter}_{kb}_{d2}")
                            nc.vector.tensor_add(st[:], p[:],
                                                 vb_bcast[:, d2 * 512:(d2 + 1) * 512])
                            nc.sync.dma_start(
                                v_d[(k0 + kb) * 128:(k0 + kb + 1) * 128,
                                    d2 * 512:(d2 + 1) * 512],
                                st[:])
                if debug:
                    nc.sync.dma_start(dsemb[:, :], semb_pc[:].bitcast(F32))
                    nc.sync.dma_start(dkbias[:, :], kbias_sb[:])
                    nc.sync.dma_start(dvb[:, :], vb_bcast[0:1, :])

            # ================= Stage 2: attention per head =================
            with (
                tc.tile_pool(name="attn", bufs=2) as attn,
                tc.tile_pool(name="kvin", bufs=2) as kvin,
                tc.tile_pool(name="rlp", bufs=2) as rlp,
                tc.tile_pool(name="ps_s", bufs=2, space="PSUM") as ps_s,
                tc.tile_pool(name="ps_l", bufs=2, space="PSUM") as ps_l,
                tc.tile_pool(name="ps_c", bufs=1, space="PSUM") as ps_c,
            ):
                for h in range(NH):
                    PT = attn.tile([128, KC, SQ], F32R, tag="PT")
                    lsum = ps_l.tile([1, SQ], F32, tag="lsum")
                    ctx_ps = [ps_c.tile([128, SQ], F32, tag=f"ctx{dv}", name=f"ctxps{h}_{dv}")
                              for dv in range(4)]
                    kt = None
                    for kc in range(KC):
                        if kc % 4 == 0:
                            kt = kvin.tile([128, 4, 512], F32R, tag="ktin")
                            nc.sync.dma_start(
                                kt,
                                kT_d[4 * h:4 * h + 4, :, kc * 128:kc * 128 + 512]
                                .rearrange("c p k -> p c k"))
                        ps = ps_s.tile([128, SQ], F32, tag="st")
                        for dq in range(4):
                            nc.tensor.matmul(
                                ps[:],
                                kt[:, dq, (kc % 4) * 128:(kc % 4) * 128 + 128],
                                qT_sb[:, 4 * h + dq, :],
                                start=(dq == 0), stop=(dq == 3))
                        bias_ap = mb_sb[:, kc:kc + 1] if h == 0 else zb_sb[:, 0:1]
                        nc.scalar.activation(PT[:, kc, :], ps[:], AF.Exp, bias=bias_ap)
                        if debug and h == 0 and kc == 0:
                            nc.sync.dma_start(dPT[:, :], PT[:, 0, :].bitcast(F32))
                            nc.sync.dma_start(dKT[:, :, :], kt[:].bitcast(F32))
                            stdbg = rlp.tile([128, 512], F32, tag="stdbg", bufs=1)
                            nc.vector.tensor_copy(stdbg[:], ps[:])
                            nc.sync.dma_start(dST[:, :], stdbg[:])
                        nc.tensor.matmul(lsum[:], ones_sb[:], PT[:, kc, :],
                                         start=(kc == 0), stop=(kc == KC - 1),
                                         skip_group_check=True)
                        vt = kvin.tile([128, HD], F32R, tag="vtin")
                        nc.sync.dma_start(vt, v_d[kc * 128:(kc + 1) * 128,
                                                  h * HD:(h + 1) * HD])
                        for dv in range(4):
                            nc.tensor.matmul(ctx_ps[dv][:],
                                             vt[:, dv * 128:(dv + 1) * 128],
                                             PT[:, kc, :],
                                             start=(kc == 0), stop=(kc == KC - 1),
                                             skip_group_check=True)
                    # softmax denominators -> broadcast reciprocal
                    rl = rlp.tile([1, SQ], F32, tag="rl")
                    nc.vector.reciprocal(rl[:], lsum[:])
                    nc.sync.dma_start(l_scr[h:h + 1, :], rl[:])
                    if debug:
                        nc.sync.dma_start(dl[h:h + 1, :], rl[:])
                    rl_b = rlp.tile([128, SQ], F32, tag="rlb")
                    nc.sync.dma_start(rl_b,
                                      bass.AP(tensor=l_scr.tensor,
                                              offset=l_scr.offset + h * SQ,
                                              ap=[[0, 128], [1, SQ]]))
                    for dv in range(4):
                        nc.vector.tensor_mul(ctx_sb[:, 4 * h + dv, :], ctx_ps[dv][:], rl_b[:])

            # ================= Stage 4: out-proj (natural layout) + residual + LN ====
            if debug:
                nc.sync.dma_start(dctx[:, :], ctx_sb[:, 0, :].bitcast(F32))
            with (
                tc.tile_pool(name="s4", bufs=2) as s4,
                tc.tile_pool(name="ps4", bufs=3, space="PSUM") as ps4,
            ):
                wo_sb = s4.tile([128, HC, H], F32R, tag="wo", bufs=1)
                nc.sync.dma_start(wo_sb, woT.rearrange("(c p) d -> p c d", p=128))
                lnw_b = s4.tile([128, H], F32, tag="lnwb", bufs=1)
                nc.sync.dma_start(lnw_b, bass.AP(tensor=lnw, offset=0, ap=[[0, 128], [1, H]]))
                lnb_b = s4.tile([128, H], F32, tag="lnbb", bufs=1)
                nc.sync.dma_start(lnb_b, bass.AP(tensor=lnb, offset=0, ap=[[0, 128], [1, H]]))
                for qb in range(QB):
                    xq_f = s4.tile([128, H], F32, tag="xqf")
                    nc.sync.dma_start(xq_f, xq[qb * 128:(qb + 1) * 128, :])
                    res_f = s4.tile([128, H], F32, tag="resf")
                    for h2 in range(H // 512):
                        p = ps4.tile([128, 512], F32, tag="pout", name=f"po{qb}_{h2}")
                        for dc in range(HC):
                            nc.tensor.matmul(p[:], ctx_sb[:, dc, qb * 128:(qb + 1) * 128],
                                             wo_sb[:, dc, h2 * 512:(h2 + 1) * 512],
                                             start=(dc == 0), stop=(dc == HC - 1))
                        # fused: out + residual
                        nc.vector.tensor_add(res_f[:, h2 * 512:(h2 + 1) * 512], p[:],
                                             xq_f[:, h2 * 512:(h2 + 1) * 512])
                    if debug and qb == 0:
                        nc.sync.dma_start(doutT[:, :], res_f[:, 0:512])
                    # LayerNorm via bn_stats
                    stats = s4.tile([128, 2, 6], F32, tag="stats")
                    for h2 in range(H // 512):
                        nc.vector.bn_stats(stats[:, h2, :],
                                           res_f[:, h2 * 512:(h2 + 1) * 512])
                    mv = s4.tile([128, 2], F32, tag="mv")
                    nc.vector.bn_aggr(mv[:], stats[:])
                    sd_t = s4.tile([128, 1], F32, tag="sdt")
                    nc.scalar.activation(sd_t[:], mv[:, 1:2], AF.Sqrt, bias=eps_sb[:])
                    rstd = s4.tile([128, 1], F32, tag="rstd")
                    nc.vector.reciprocal(rstd[:], sd_t[:])
                    norm = s4.tile([128, H], F32, tag="norm")
                    nc.vector.tensor_scalar(norm[:], res_f[:], mv[:, 0:1], rstd[:],
                                            ALU.subtract, ALU.mult)
                    scl = s4.tile([128, H], F32, tag="scl")
                    nc.vector.tensor_mul(scl[:], norm[:], lnw_b[:])
                    fin = s4.tile([128, H], F32, tag="fin")
                    nc.vector.tensor_add(fin[:], scl[:], lnb_b[:])
                    nc.sync.dma_start(out[qb * 128:(qb + 1) * 128, :], fin[:])

    nc.compile()
    return nc


_CACHED_NC = {}


def _get_nc(debug=False):
    if debug not in _CACHED_NC:
        _CACHED_NC[debug] = build_program(debug)
    return _CACHED_NC[debug]


def _prep_inputs(inputs, static_data, base_mask, Wq, Wk, Wv, Wo, Ws, bs, ln_w, ln_b):
    f32 = np.float32
    common = {
        "x": np.ascontiguousarray(inputs, f32),
        "wqT": np.ascontiguousarray(np.asarray(Wq, f32).T),
        "wkT": np.ascontiguousarray(np.asarray(Wk, f32).T),
        "wvT": np.ascontiguousarray(np.asarray(Wv, f32).T),
        "woT": np.ascontiguousarray(np.asarray(Wo, f32).T),
        "wsT": np.ascontiguousarray(np.asarray(Ws, f32).T),
        "sdat": np.ascontiguousarray(np.asarray(static_data, f32).reshape(DS, 1)),
        "bsv": np.ascontiguousarray(bs, f32),
        "mbias": np.where(np.asarray(base_mask, bool), 0.0, -1e30).astype(f32),
        "onescol": np.ones((128, 1), f32),
        "identd": np.eye(128, dtype=f32),
        "lnw": np.ascontiguousarray(ln_w, f32),
        "lnb": np.ascontiguousarray(ln_b, f32),
    }
    x = common["x"]
    in_maps = []
    for c in range(NC):
        m = dict(common)
        m["xq"] = np.ascontiguousarray(x[c * SQ:(c + 1) * SQ, :])
        in_maps.append(m)
    return in_maps


def kernel_run(trace=False, debug=False, **inputs):
    nc = _get_nc(debug)
    in_maps = _prep_inputs(**inputs)
    res = run_bass_kernel_spmd(nc, in_maps, core_ids=list(range(NC)), trace=trace)
    outp = np.concatenate([res.results[c]["out"] for c in range(NC)], axis=0)
    return outp, res


def kernel(**inputs):
    outp, _ = kernel_run(trace=False, **inputs)
    return outp


# revision 16
# speedup vs baseline: 1.0141x; 1.0141x over previous
"""Trainium2 Bass kernel for nn_AttentionBlock (S=4096, H=1024, NH=2, DS=64).

Strategy: sequence parallelism over queries (512 per core on 8 cores).
K/V projections are replicated on every core (cheaper than collectives here).
All matmuls run in float32r (full PE rate, ~1e-4 relative rounding).
"""

import math
import sys

sys.path.insert(0, "/opt/trn_rl_repo")

import numpy as np

import concourse.bass as bass
import concourse.mybir as mybir
import concourse.tile as tile
from concourse import bacc
from concourse.bass_utils import run_bass_kernel_spmd

S, H, NH, DS = 4096, 1024, 2, 64
HD = H // NH            # 512
NC = 8                  # cores
SQ = S // NC            # 512 queries per core
EPS = 1e-5
F32 = mybir.dt.float32
F32R = mybir.dt.float32r
AF = mybir.ActivationFunctionType
ALU = mybir.AluOpType

KC = S // 128           # 32 key chunks of 128
HC = H // 128           # 8 hidden chunks of 128
QB = SQ // 128          # 4 query chunks of 128


def build_program(debug=False):
    nc = bacc.Bacc("TRN2", target_bir_lowering=False, debug=False, num_devices=NC)

    # ---- DRAM I/O ----
    x = nc.dram_tensor("x", [S, H], F32, kind="ExternalInput")
    xq = nc.dram_tensor("xq", [SQ, H], F32, kind="ExternalInput")
    wqT = nc.dram_tensor("wqT", [H, H], F32R, kind="ExternalInput")
    wkT = nc.dram_tensor("wkT", [H, H], F32R, kind="ExternalInput")
    wvT = nc.dram_tensor("wvT", [H, H], F32R, kind="ExternalInput")
    woT = nc.dram_tensor("woT", [H, H], F32R, kind="ExternalInput")
    wsT = nc.dram_tensor("wsT", [DS, H], F32R, kind="ExternalInput")
    sdat = nc.dram_tensor("sdat", [DS, 1], F32R, kind="ExternalInput")
    bsv = nc.dram_tensor("bsv", [H], F32, kind="ExternalInput")
    mbias = nc.dram_tensor("mbias", [S], F32, kind="ExternalInput")
    onescol = nc.dram_tensor("onescol", [128, 1], F32R, kind="ExternalInput")
    identd = nc.dram_tensor("identd", [128, 128], F32R, kind="ExternalInput")
    lnw = nc.dram_tensor("lnw", [H], F32, kind="ExternalInput")
    lnb = nc.dram_tensor("lnb", [H], F32, kind="ExternalInput")
    out = nc.dram_tensor("out", [SQ, H], F32, kind="ExternalOutput")
    if debug:
        dsemb = nc.dram_tensor("dsemb", [128, HC], F32, kind="ExternalOutput")
        dkbias = nc.dram_tensor("dkbias", [128, HC], F32, kind="ExternalOutput")
        dvb = nc.dram_tensor("dvb", [1, H], F32, kind="ExternalOutput")
        dxT = nc.dram_tensor("dxT", [128, 512], F32, kind="ExternalOutput")
        dqT = nc.dram_tensor("dqT", [128, 512], F32, kind="ExternalOutput")
        dPT = nc.dram_tensor("dPT", [128, 512], F32, kind="ExternalOutput")
        dl = nc.dram_tensor("dl", [NH, SQ], F32, kind="ExternalOutput")
        dctx = nc.dram_tensor("dctx", [128, 512], F32, kind="ExternalOutput")
        doutT = nc.dram_tensor("doutT", [128, 512], F32, kind="ExternalOutput")
        dKT = nc.dram_tensor("dKT", [128, 4, 512], F32, kind="ExternalOutput")
        dST = nc.dram_tensor("dST", [128, 512], F32, kind="ExternalOutput")

    inv_sqrt_hd = 1.0 / math.sqrt(HD)

    with tile.TileContext(nc) as tc:
        with (
            tc.tile_pool(name="consts", bufs=1) as consts,
            tc.tile_pool(name="persist", bufs=1) as persist,
            tc.tile_pool(name="dram", bufs=1, space="DRAM") as dram,
        ):
            # ---- constants ----
            ident = consts.tile([128, 128], F32R)
            nc.sync.dma_start(ident, identd[:, :])
            ones_sb = consts.tile([128, 1], F32R)
            nc.sync.dma_start(ones_sb, onescol[:, :])
            mb_sb = consts.tile([128, KC], F32)
            nc.sync.dma_start(mb_sb, mbias.rearrange("(c p) -> p c", p=128))
            zb_sb = consts.tile([128, 1], F32)
            nc.vector.memset(zb_sb, 0.0)
            eps_sb = consts.tile([128, 1], F32)
            nc.vector.memset(eps_sb, EPS)
            bs_row = consts.tile([1, H], F32)
            nc.sync.dma_start(bs_row, bsv.rearrange("d -> () d"))
            wsT_sb = consts.tile([DS, H], F32R)
            nc.sync.dma_start(wsT_sb, wsT[:, :])
            sd_sb = consts.tile([DS, 1], F32R)
            nc.sync.dma_start(sd_sb, sdat[:, :])

            # persistent across stages
            qT_sb = persist.tile([128, HC, SQ], F32R)      # Q^T/sqrt(hd): [d, q]
            ctx_sb = persist.tile([128, HC, SQ], F32R)     # ctx^T/l: [d, q]
            semb_pc = persist.tile([128, HC], F32R)        # static embedding [hin_p, hc]
            kbias_sb = persist.tile([128, HC], F32)

            # DRAM scratch
            kT_d = dram.tile([HC, 128, S], F32R)           # K^T as [dc, d_in_chunk, k]
            v_d = dram.tile([S, H], F32R)                  # V natural [k, d]
            vb_scr = dram.tile([H], F32)
            semb_scr = dram.tile([H], F32R)
            kb_scr = dram.tile([H], F32)
            l_scr = dram.tile([NH, SQ], F32)

            # ================= Stage 1: projections (eight key-blocks) =================
            SH = S // 8      # 512 keys per block
            KH = SH // 128   # 4 key chunks per block
            with (
                tc.tile_pool(name="xtp", bufs=2) as xtp,
                tc.tile_pool(name="w1", bufs=2) as w1,
                tc.tile_pool(name="ps1", bufs=3, space="PSUM") as ps1,
                tc.tile_pool(name="pst", bufs=3, space="PSUM") as pst,
                tc.tile_pool(name="psb", bufs=2, space="PSUM") as psb,
            ):
                vb_bcast = xtp.tile([128, H], F32, tag="vbb", bufs=1)
                # --- semb = Ws @ static + bs (row layout, then roundtrip) ---
                semb_row = xtp.tile([1, H], F32R, tag="srow", bufs=1)
                for d2 in range(H // 512):
                    p = psb.tile([1, 512], F32, tag="pbias", name=f"sembp{d2}")
                    nc.tensor.matmul(p[:], sd_sb[:], wsT_sb[:, d2 * 512:(d2 + 1) * 512],
                                     start=True, stop=True)
                    nc.vector.tensor_add(semb_row[:, d2 * 512:(d2 + 1) * 512], p[:],
                                         bs_row[:, d2 * 512:(d2 + 1) * 512])
                nc.sync.dma_start(semb_scr.rearrange("d -> () d"), semb_row[:])
                nc.sync.dma_start(semb_pc, semb_scr.rearrange("(c p) -> p c", p=128))

                # --- xq transpose + Q^T (scaled) first: shortens path to attention ---
                wq_sb = w1.tile([128, HC, H], F32R, tag="w", name="wq")
                nc.sync.dma_start(wq_sb, wqT.rearrange("(c p) d -> p c d", p=128))
                xqT_sb = xtp.tile([128, HC, SQ], F32R, tag="xT", name="xqT")
                for qb in range(QB):
                    xin = xtp.tile([128, H], F32R, tag="xin", name=f"xqin{qb}")
                    nc.sync.dma_start(xin, xq[qb * 128:(qb + 1) * 128, :].bitcast(F32R))
                    for hc in range(HC):
                        pt = pst.tile([128, 128], F32R, tag="ptr", name=f"qtr{qb}_{hc}")
                        nc.tensor.transpose(pt[:], xin[:, hc * 128:(hc + 1) * 128], ident[:])
                        nc.any.tensor_copy(xqT_sb[:, hc, qb * 128:(qb + 1) * 128], pt[:])
                for dc in range(HC):
                    p = ps1.tile([128, SQ], F32, tag="pproj", name=f"qp{dc}")
                    for hc in range(HC):
                        nc.tensor.matmul(p[:], wq_sb[:, hc, dc * 128:(dc + 1) * 128],
                                         xqT_sb[:, hc, :],
                                         start=(hc == 0), stop=(hc == HC - 1))
                    nc.scalar.mul(qT_sb[:, dc, :], p[:], inv_sqrt_hd)
                if debug:
                    nc.sync.dma_start(dqT[:, :], qT_sb[:, 0, :].bitcast(F32))

                wk_sb = w1.tile([128, HC, H], F32R, tag="w", name="wk")
                nc.sync.dma_start(wk_sb, wkT.rearrange("(c p) d -> p c d", p=128))
                wv_sb = w1.tile([128, HC, H], F32R, tag="w2", name="wv", bufs=1)
                nc.sync.dma_start(wv_sb, wvT.rearrange("(c p) d -> p c d", p=128))

                for quarter in range(8):
                    k0 = quarter * KH       # first 128-chunk of this quarter
                    # --- transpose x rows of this quarter -> xT_sb [128, HC, SH] ---
                    xT_sb = xtp.tile([128, HC, SH], F32R, tag="xT", name=f"xT{quarter}")
                    for kb in range(KH):
                        xin = xtp.tile([128, H], F32R, tag="xin",
                                         name=f"xin{quarter}_{kb}")
                        nc.sync.dma_start(xin, x[(k0 + kb) * 128:(k0 + kb + 1) * 128, :]
                                          .bitcast(F32R))
                        for hc in range(HC):
                            pt = pst.tile([128, 128], F32R, tag="ptr",
                                          name=f"ptr{quarter}_{kb}_{hc}")
                            nc.tensor.transpose(pt[:], xin[:, hc * 128:(hc + 1) * 128],
                                                ident[:])
                            nc.any.tensor_copy(xT_sb[:, hc, kb * 128:(kb + 1) * 128], pt[:])

                    if debug and quarter == 0:
                        nc.sync.dma_start(dxT[:, :], xT_sb[:, 0, 0:512].bitcast(F32))
                    # --- K^T for this quarter's key columns ---
                    if quarter == 0:
                        kb_row = xtp.tile([1, H], F32, tag="krow", bufs=1)
                        for d2 in range(H // 512):
                            p = psb.tile([1, 512], F32, tag="pbias", name=f"kbp{d2}")
                            for hc in range(HC):
                                nc.tensor.matmul(p[:], semb_pc[:, hc:hc + 1],
                                                 wk_sb[:, hc, d2 * 512:(d2 + 1) * 512],
                                                 start=(hc == 0), stop=(hc == HC - 1))
                            nc.vector.tensor_copy(kb_row[:, d2 * 512:(d2 + 1) * 512], p[:])
                        nc.sync.dma_start(kb_scr.rearrange("d -> () d"), kb_row[:])
                        nc.sync.dma_start(kbias_sb, kb_scr.rearrange("(c p) -> p c", p=128))
                        vb_row = xtp.tile([1, H], F32, tag="vrow", bufs=1)
                        for d2 in range(H // 512):
                            p = psb.tile([1, 512], F32, tag="pbias", name=f"vbp{d2}")
                            for hc in range(HC):
                                nc.tensor.matmul(p[:], semb_pc[:, hc:hc + 1],
                                                 wv_sb[:, hc, d2 * 512:(d2 + 1) * 512],
                                                 start=(hc == 0), stop=(hc == HC - 1))
                            nc.vector.tensor_copy(vb_row[:, d2 * 512:(d2 + 1) * 512], p[:])
                        nc.sync.dma_start(vb_scr.rearrange("d -> () d"), vb_row[:])
                        nc.sync.dma_start(vb_bcast,
                                          bass.AP(tensor=vb_scr.tensor, offset=vb_scr.offset,
                                                  ap=[[0, 128], [1, H]]))
                    for dc in range(HC):
                        for k2 in range(SH // 512):
                            p = ps1.tile([128, 512], F32, tag="pproj",
                                         name=f"kp{quarter}_{dc}_{k2}")
                            for hc in range(HC):
                                nc.tensor.matmul(p[:], wk_sb[:, hc, dc * 128:(dc + 1) * 128],
                                                 xT_sb[:, hc, k2 * 512:(k2 + 1) * 512],
                                                 start=(hc == 0), stop=(hc == HC - 1))
                            st = xtp.tile([128, 512], F32R, tag="kst",
                                            name=f"kst{quarter}_{dc}_{k2}")
                            nc.scalar.activation(st[:], p[:], AF.Identity,
                                                 bias=kbias_sb[:, dc:dc + 1])
                            nc.sync.dma_start(
                                kT_d[dc, :,
                                     quarter * SH + k2 * 512:quarter * SH + (k2 + 1) * 512],
                                st[:])

                    # --- V for this quarter's key rows ---
                    for kb in range(KH):
                        for d2 in range(H // 512):
                            p = ps1.tile([128, 512], F32, tag="pproj",
                                         name=f"vp{quarter}_{kb}_{d2}")
                            for hc in range(HC):
                                nc.tensor.matmul(p[:], xT_sb[:, hc, kb * 128:(kb + 1) * 128],
                                                 wv_sb[:, hc, d2 * 512:(d2 + 1) * 512],
                                                 start=(hc == 0), stop=(hc == HC - 1))
                            st = xtp.tile([128, 512], F32R, tag="vst",
                                            name=f"vst{quarter}_{kb}_{d2}")
                            nc.vector.tensor_add(st[:], p[:],
                                                 vb_bcast[:, d2 * 512:(d2 + 1) * 512])
                            nc.sync.dma_start(
                                v_d[(k0 + kb) * 128:(k0 + kb + 1) * 128,
                                    d2 * 512:(d2 + 1) * 512],
                                st[:])
                if debug:
                    nc.sync.dma_start(dsemb[:, :], semb_pc[:].bitcast(F32))
                    nc.sync.dma_start(dkbias[:, :], kbias_sb[:])
                    nc.sync.dma_start(dvb[:, :], vb_bcast[0:1, :])

            # ========== Stage 2+4: attention per head, out-proj, residual, LN ==========
            with (
                tc.tile_pool(name="attn", bufs=1) as attn,
                tc.tile_pool(name="kvin", bufs=2) as kvin,
                tc.tile_pool(name="rlp", bufs=2) as rlp,
                tc.tile_pool(name="s4", bufs=2) as s4,
                tc.tile_pool(name="ps_s", bufs=2, space="PSUM") as ps_s,
                tc.tile_pool(name="ps_misc", bufs=2, space="PSUM") as ps_misc,
                tc.tile_pool(name="ps_c", bufs=1, space="PSUM") as ps_c,
            ):
                # prefetch stage-4 constants while attention runs
                wo_sb = s4.tile([128, HC, H], F32R, tag="wo", bufs=1)
                nc.sync.dma_start(wo_sb, woT.rearrange("(c p) d -> p c d", p=128))
                lnw_b = s4.tile([128, H], F32, tag="lnwb", bufs=1)
                nc.sync.dma_start(lnw_b, bass.AP(tensor=lnw, offset=0, ap=[[0, 128], [1, H]]))
                lnb_b = s4.tile([128, H], F32, tag="lnbb", bufs=1)
                nc.sync.dma_start(lnb_b, bass.AP(tensor=lnb, offset=0, ap=[[0, 128], [1, H]]))

                for h in range(NH):
                    PT = attn.tile([128, KC, SQ], F32R, tag="PT", name=f"PT{h}")
                    lsum = ps_misc.tile([1, SQ], F32, tag="misc", name=f"lsum{h}")
                    ctx_ps = [ps_c.tile([128, SQ], F32, tag=f"ctx{dv}", name=f"ctxps{h}_{dv}")
                              for dv in range(4)]
                    kt = None
                    for kc in range(KC):
                        if kc % 4 == 0:
                            kt = kvin.tile([128, 4, 512], F32R, tag="ktin",
                                           name=f"kt{h}_{kc}")
                            nc.sync.dma_start(
                                kt,
                                kT_d[4 * h:4 * h + 4, :, kc * 128:kc * 128 + 512]
                                .rearrange("c p k -> p c k"))
                        ps = ps_s.tile([128, SQ], F32, tag="st", name=f"st{h}_{kc}")
                        for dq in range(4):
                            nc.tensor.matmul(
                                ps[:],
                                kt[:, dq, (kc % 4) * 128:(kc % 4) * 128 + 128],
                                qT_sb[:, 4 * h + dq, :],
                                start=(dq == 0), stop=(dq == 3))
                        bias_ap = mb_sb[:, kc:kc + 1] if h == 0 else zb_sb[:, 0:1]
                        nc.scalar.activation(PT[:, kc, :], ps[:], AF.Exp, bias=bias_ap)
                        if debug and h == 0 and kc == 0:
                            nc.sync.dma_start(dPT[:, :], PT[:, 0, :].bitcast(F32))
                            nc.sync.dma_start(dKT[:, :, :], kt[:].bitcast(F32))
                            stdbg = rlp.tile([128, 512], F32, tag="stdbg", bufs=1)
                            nc.vector.tensor_copy(stdbg[:], ps[:])
                            nc.sync.dma_start(dST[:, :], stdbg[:])
                        nc.tensor.matmul(lsum[:], ones_sb[:], PT[:, kc, :],
                                         start=(kc == 0), stop=(kc == KC - 1),
                                         skip_group_check=True)
                        vt = kvin.tile([128, HD], F32R, tag="vtin", name=f"vt{h}_{kc}")
                        nc.sync.dma_start(vt, v_d[kc * 128:(kc + 1) * 128,
                                                  h * HD:(h + 1) * HD])
                        for dv in range(4):
                            nc.tensor.matmul(ctx_ps[dv][:],
                                             vt[:, dv * 128:(dv + 1) * 128],
                                             PT[:, kc, :],
                                             start=(kc == 0), stop=(kc == KC - 1),
                                             skip_group_check=True)
                    # softmax denominators -> broadcast reciprocal
                    rl = rlp.tile([1, SQ], F32, tag="rl", name=f"rl{h}")
                    nc.vector.reciprocal(rl[:], lsum[:])
                    nc.sync.dma_start(l_scr[h:h + 1, :], rl[:])
                    if debug:
                        nc.sync.dma_start(dl[h:h + 1, :], rl[:])
                    rl_b = rlp.tile([128, SQ], F32, tag="rlb", name=f"rlb{h}")
                    nc.sync.dma_start(rl_b,
                                      bass.AP(tensor=l_scr.tensor,
                                              offset=l_scr.offset + h * SQ,
                                              ap=[[0, 128], [1, SQ]]))
                    for dv in range(4):
                        nc.vector.tensor_mul(ctx_sb[:, 4 * h + dv, :], ctx_ps[dv][:], rl_b[:])

                # ---- out-proj (natural layout) + fused residual + LN ----
                if debug:
                    nc.sync.dma_start(dctx[:, :], ctx_sb[:, 0, :].bitcast(F32))
                for qb in range(QB):
                    xq_f = s4.tile([128, H], F32, tag="xqf", name=f"xqf{qb}")
                    nc.sync.dma_start(xq_f, xq[qb * 128:(qb + 1) * 128, :])
                    res_f = s4.tile([128, H], F32, tag="resf", name=f"resf{qb}")
                    for h2 in range(H // 512):
                        p = ps_misc.tile([128, 512], F32, tag="misc", name=f"po{qb}_{h2}")
                        for dc in range(HC):
                            nc.tensor.matmul(p[:], ctx_sb[:, dc, qb * 128:(qb + 1) * 128],
                                             wo_sb[:, dc, h2 * 512:(h2 + 1) * 512],
                                             start=(dc == 0), stop=(dc == HC - 1))
                        nc.vector.tensor_add(res_f[:, h2 * 512:(h2 + 1) * 512], p[:],
                                             xq_f[:, h2 * 512:(h2 + 1) * 512])
                    if debug and qb == 0:
                        nc.sync.dma_start(doutT[:, :], res_f[:, 0:512])
                    # LayerNorm via bn_stats
                    stats = s4.tile([128, 2, 6], F32, tag="stats", name=f"stats{qb}")
                    for h2 in range(H // 512):
                        nc.vector.bn_stats(stats[:, h2, :],
                                           res_f[:, h2 * 512:(h2 + 1) * 512])
                    mv = s4.tile([128, 2], F32, tag="mv", name=f"mv{qb}")
                    nc.vector.bn_aggr(mv[:], stats[:])
                    sd_t = s4.tile([128, 1], F32, tag="sdt", name=f"sdt{qb}")
                    nc.scalar.activation(sd_t[:], mv[:, 1:2], AF.Sqrt, bias=eps_sb[:])
                    rstd = s4.tile([128, 1], F32, tag="rstd", name=f"rstd{qb}")
                    nc.vector.reciprocal(rstd[:], sd_t[:])
                    norm = s4.tile([128, H], F32, tag="norm", name=f"norm{qb}", bufs=1)
                    nc.vector.tensor_scalar(norm[:], res_f[:], mv[:, 0:1], rstd[:],
                                            ALU.subtract, ALU.mult)
                    scl = s4.tile([128, H], F32, tag="scl", name=f"scl{qb}", bufs=1)
                    nc.vector.tensor_mul(scl[:], norm[:], lnw_b[:])
                    fin = s4.tile([128, H], F32, tag="fin", name=f"fin{qb}")
                    nc.vector.tensor_add(fin[:], scl[:], lnb_b[:])
                    nc.sync.dma_start(out[qb * 128:(qb + 1) * 128, :], fin[:])

    nc.compile()
    return nc


_CACHED_NC = {}


def _get_nc(debug=False):
    if debug not in _CACHED_NC:
        _CACHED_NC[debug] = build_program(debug)
    return _CACHED_NC[debug]


def _prep_inputs(inputs, static_data, base_mask, Wq, Wk, Wv, Wo, Ws, bs, ln_w, ln_b):
    f32 = np.float32
    common = {
        "x": np.ascontiguousarray(inputs, f32),
        "wqT": np.ascontiguousarray(np.asarray(Wq, f32).T),
        "wkT": np.ascontiguousarray(np.asarray(Wk, f32).T),
        "wvT": np.ascontiguousarray(np.asarray(Wv, f32).T),
        "woT": np.ascontiguousarray(np.asarray(Wo, f32).T),
        "wsT": np.ascontiguousarray(np.asarray(Ws, f32).T),
        "sdat": np.ascontiguousarray(np.asarray(static_data, f32).reshape(DS, 1)),
        "bsv": np.ascontiguousarray(bs, f32),
        "mbias": np.where(np.asarray(base_mask, bool), 0.0, -1e30).astype(f32),
        "onescol": np.ones((128, 1), f32),
        "identd": np.eye(128, dtype=f32),
        "lnw": np.ascontiguousarray(ln_w, f32),
        "lnb": np.ascontiguousarray(ln_b, f32),
    }
    x = common["x"]
    in_maps = []
    for c in range(NC):
        m = dict(common)
        m["xq"] = np.ascontiguousarray(x[c * SQ:(c + 1) * SQ, :])
        in_maps.append(m)
    return in_maps


def kernel_run(trace=False, debug=False, **inputs):
    nc = _get_nc(debug)
    in_maps = _prep_inputs(**inputs)
    res = run_bass_kernel_spmd(nc, in_maps, core_ids=list(range(NC)), trace=trace)
    outp = np.concatenate([res.results[c]["out"] for c in range(NC)], axis=0)
    return outp, res


def kernel(**inputs):
    outp, _ = kernel_run(trace=False, **inputs)
    return outp
